# revision 3
# baseline (speedup 1.0000x reference)
"""GCN layer (4-relation message passing) on 8 Trainium2 NeuronCores.

out = sum_r (A_r @ inp) @ W_r + sum_r b_r,  A_r in COO form (dst, src, val).

Sharding: edges sharded by dst range; core c owns dst in [c*12500, (c+1)*12500).

Host stages, per 128-edge block, only the raw values (pure placement, no
host arithmetic):
  - msg slab [128, T*64]  bf16: rows inp[src] for each edge slot
  - meta slab [128, 2*T]  bf16: col 2t = dst-within-window, col 2t+1 = val

Device does all FLOPs. Per block b of cell (window w, rel r):
  onehot_b[p, j] = (iota[j] == dst_p) * val_p        one fused tensor_scalar
                                                     (DVE, some on GpSimd)
  PE: psum[64f, 256][:, r*64:] += msg_b^T @ onehot_b (edge_val scaling and the
                                                     dst segment-sum live in
                                                     this matmul)
Per window: ACT copies psum -> aggsb bf16 [64, 256]; 4x PE stage-2
  po[64 dst, 64 outf] += aggsb_r^T @ W_r; ACT copies po into outsb.
Host: concat core slices, add summed bias.
"""

import math
from contextlib import ExitStack

import numpy as np

import concourse.bass as bass
import concourse.tile as tile
from concourse import bacc, mybir
from concourse.bass_utils import run_bass_kernel_spmd

# problem constants
N_NODES = 100000
N_REL = 4
N_EDGES = 1600000
IN_SIZE = 64
OUT_SIZE = 64

N_CORES = 8
NPC = N_NODES // N_CORES  # nodes (dst) per core
P = 128                   # partitions / edges per block
W = 64                    # dst-window width (psum tile col group)
GW = 4                    # windows per msg-slab DMA chunk
GPS_EVERY = 5             # every GPS_EVERY-th block's onehot built on GpSimd

F32 = mybir.dt.float32
BF16 = mybir.dt.bfloat16


def _np_bf16():
    import ml_dtypes
    return ml_dtypes.bfloat16


def _host_prep(inp, src, dst, edge_val):
    """Bucket/pad edges per (core, window, rel); build msg + meta slabs."""
    n_win = math.ceil(NPC / W)
    ncell = n_win * N_REL
    srcf = src.reshape(-1).astype(np.int64)
    dstf = dst.reshape(-1).astype(np.int64)
    valf = edge_val.reshape(-1).astype(np.float32)
    rel = np.repeat(np.arange(N_REL, dtype=np.int64), src.shape[1])

    core = dstf // NPC
    dloc = dstf % NPC
    win = dloc // W
    wloc = dloc % W
    cell = win * N_REL + rel
    key = core * ncell + cell

    counts = np.bincount(key, minlength=N_CORES * ncell).reshape(
        N_CORES, ncell)
    B = np.maximum((counts.max(axis=0) + P - 1) // P, 1).astype(np.int64)
    starts = np.zeros(ncell + 1, dtype=np.int64)
    np.cumsum(B, out=starts[1:])
    T = int(starts[-1])

    edt = _np_bf16()
    msg_all = np.zeros((N_CORES, P, T, IN_SIZE), dtype=edt)
    meta_all = np.zeros((N_CORES, P, T, 2), dtype=np.float32)

    order = np.argsort(key, kind="stable")
    grp_start = np.zeros(N_CORES * ncell, dtype=np.int64)
    np.cumsum(counts.reshape(-1)[:-1], out=grp_start[1:])
    j = np.arange(len(order), dtype=np.int64) - grp_start[key[order]]
    t_col = starts[cell[order]] + (j // P)
    p_row = j % P
    c_ord = core[order]
    msg_all[c_ord, p_row, t_col] = inp[srcf[order]].astype(edt)
    meta_all[c_ord, p_row, t_col, 0] = wloc[order].astype(np.float32)
    meta_all[c_ord, p_row, t_col, 1] = valf[order]

    return n_win, B, starts, T, msg_all, meta_all


_PROG_CACHE = {}


def _build_program(n_win, starts, T):
    key = (n_win, tuple(int(s) for s in starts), T)
    if key in _PROG_CACHE:
        return _PROG_CACHE[key]

    nc = bacc.Bacc("TRN2", target_bir_lowering=False, debug=False,
                   num_devices=N_CORES)
    wcat = nc.dram_tensor("wcat", [IN_SIZE, N_REL * OUT_SIZE], BF16,
                          kind="ExternalInput").ap()
    iotat = nc.dram_tensor("iotat", [P, W], BF16,
                           kind="ExternalInput").ap()
    emsg = nc.dram_tensor("emsg", [P, T * IN_SIZE], BF16,
                          kind="ExternalInput").ap()
    emeta = nc.dram_tensor("emeta", [P, T * 2], F32,
                           kind="ExternalInput").ap()
    n_wcol = (n_win + 1) // 2
    out = nc.dram_tensor("out", [P, n_wcol * OUT_SIZE], F32,
                         kind="ExternalOutput").ap()

    eq = mybir.AluOpType.is_equal
    mul = mybir.AluOpType.mult

    with tile.TileContext(nc) as tc, ExitStack() as ctx:
        p_const = ctx.enter_context(tc.tile_pool(name="p_const", bufs=1))
        p_meta = ctx.enter_context(tc.tile_pool(name="p_meta", bufs=1))
        p_msg = ctx.enter_context(tc.tile_pool(name="p_msg", bufs=3))
        p_oh = ctx.enter_context(tc.tile_pool(name="p_oh", bufs=10))
        p_agg = ctx.enter_context(tc.tile_pool(name="p_agg", bufs=3))
        p_c = ctx.enter_context(tc.tile_pool(name="p_c", bufs=1))
        ps_agg = ctx.enter_context(tc.tile_pool(name="ps_agg", bufs=4,
                                                space="PSUM"))
        ps_out = ctx.enter_context(tc.tile_pool(name="ps_out", bufs=2,
                                                space="PSUM"))

        wt = p_const.tile([IN_SIZE, N_REL * OUT_SIZE], BF16)
        nc.sync.dma_start(wt[:], wcat[:])
        iot = p_const.tile([P, W], BF16)
        nc.sync.dma_start(iot[:], iotat[:])
        meta = p_meta.tile([P, T * 2], F32)
        nc.scalar.dma_start(meta[:], emeta[:])
        outsb = p_c.tile([P, n_wcol * OUT_SIZE], F32)
        if n_win % 2:
            nc.vector.memset(outsb[W:P, (n_win // 2) * OUT_SIZE:], 0.0)

        bg_max = max(
            int(starts[min(w0 + GW, n_win) * N_REL] - starts[w0 * N_REL])
            for w0 in range(0, n_win, GW))
        blk = 0  # global block counter for engine round-robin
        for w0 in range(0, n_win, GW):
            w1 = min(w0 + GW, n_win)
            t0, t1 = int(starts[w0 * N_REL]), int(starts[w1 * N_REL])
            bg = t1 - t0
            mt = p_msg.tile([P, bg_max * IN_SIZE], BF16, tag="msg")
            nc.sync.dma_start(mt[:, :bg * IN_SIZE],
                              emsg[:, t0 * IN_SIZE:t1 * IN_SIZE])
            for w in range(w0, w1):
                ps = ps_agg.tile([IN_SIZE, N_REL * W], F32)
                for r in range(N_REL):
                    c2 = w * N_REL + r
                    b0, b1 = int(starts[c2]) - t0, int(starts[c2 + 1]) - t0
                    for b in range(b0, b1):
                        oh = p_oh.tile([P, W], BF16, tag="oh")
                        tg = int(starts[c2]) + (b - b0)
                        eng = (nc.gpsimd if (blk % GPS_EVERY == GPS_EVERY - 1)
                               else nc.vector)
                        eng.tensor_scalar(
                            oh[:], iot[:],
                            meta[:, 2 * tg:2 * tg + 1],
                            meta[:, 2 * tg + 1:2 * tg + 2],
                            op0=eq, op1=mul)
                        blk += 1
                        nc.tensor.matmul(
                            out=ps[:, r * W:(r + 1) * W],
                            lhsT=mt[:, b * IN_SIZE:(b + 1) * IN_SIZE],
                            rhs=oh[:],
                            start=(b == b0), stop=(b == b1 - 1))
                agg = p_agg.tile([IN_SIZE, N_REL * W], BF16, tag="agg")
                nc.scalar.copy(agg[:], ps[:])
                po = ps_out.tile([W, OUT_SIZE], F32)
                for r in range(N_REL):
                    nc.tensor.matmul(
                        out=po[:],
                        lhsT=agg[:, r * W:(r + 1) * W],
                        rhs=wt[:, r * OUT_SIZE:(r + 1) * OUT_SIZE],
                        start=(r == 0), stop=(r == N_REL - 1))
                nc.scalar.copy(
                    outsb[(w % 2) * W:(w % 2) * W + W,
                          (w // 2) * OUT_SIZE:(w // 2 + 1) * OUT_SIZE],
                    po[:])
        nc.sync.dma_start(out[:], outsb[:])

    nc.compile()
    _PROG_CACHE[key] = nc
    return nc


def kernel(inp, src, dst, edge_val, weights, bias):
    inp = np.asarray(inp, dtype=np.float32)
    src = np.asarray(src)
    dst = np.asarray(dst)
    edge_val = np.asarray(edge_val, dtype=np.float32)
    weights = np.asarray(weights, dtype=np.float32)
    bias = np.asarray(bias, dtype=np.float32)

    n_win, B, starts, T, msg_all, meta_all = _host_prep(
        inp, src, dst, edge_val)
    nc = _build_program(n_win, starts, T)

    edt = _np_bf16()
    wcat = np.ascontiguousarray(
        weights.transpose(1, 0, 2).reshape(IN_SIZE, N_REL * OUT_SIZE)
    ).astype(edt)
    iota = np.broadcast_to(
        np.arange(W, dtype=np.float32), (P, W)).astype(edt)
    iota = np.ascontiguousarray(iota)

    in_maps = []
    for c in range(N_CORES):
        in_maps.append({
            "wcat": wcat,
            "iotat": iota,
            "emsg": msg_all[c].reshape(P, T * IN_SIZE),
            "emeta": meta_all[c].reshape(P, T * 2),
        })
    res = run_bass_kernel_spmd(nc, in_maps, list(range(N_CORES)))
    n_wcol = (n_win + 1) // 2
    parts = []
    for c in range(N_CORES):
        arr = res.results[c]["out"].reshape(2, W, n_wcol, OUT_SIZE)
        nodes = arr.transpose(2, 0, 1, 3).reshape(n_wcol * 2 * W, OUT_SIZE)
        parts.append(nodes[:NPC])
    out = np.concatenate(parts, axis=0)
    out = out + bias.sum(axis=0)
    return out.astype(np.float32)


# revision 9
# speedup vs baseline: 2.9190x; 2.9190x over previous
"""GCN layer (4-relation message passing) on 8 Trainium2 NeuronCores.

out = sum_r (A_r @ inp) @ W_r + sum_r b_r,  A_r in COO form (dst, src, val).

Sharding: edges sharded by dst range; core c owns dst in [c*12500, (c+1)*12500).
Host stages a block slab (pure placement of input values - no host
arithmetic). Edges are bucketed per (window of 32 dst nodes, relation) cell;
each cell occupies ceil(maxcount/64) half-blocks of 64 edge slots, packed
contiguously (two cells can share one 128-partition block).  Block columns
0:64 hold message rows inp[src]; columns 64:96 hold the selection matrix
O[p, j] = val_p * (j == dstloc_p).

Device, per cell: PE accumulates psum[64in, 4*32][:, r*32:] += MSG^T @ O over
the cell's segments (full 128-row blocks plus 64-row boundary halves issued
as row-tiled matmuls).  Per 4 windows: one ACT copy psum->aggG bf16
[64, 4*128]; stage-2 po[4*32 dst, 64 outf] += aggG_r^T @ W_r (4 matmuls);
one ACT copy po -> outsb.  Host: concat core slices, add summed bias.
"""

import math
from contextlib import ExitStack

import numpy as np

import concourse.bass as bass
import concourse.tile as tile
from concourse import bacc, mybir
from concourse.bass_utils import run_bass_kernel_spmd

# problem constants
N_NODES = 100000
N_REL = 4
N_EDGES = 1600000
IN_SIZE = 64
OUT_SIZE = 64

N_CORES = 8
NPC = N_NODES // N_CORES  # nodes (dst) per core
P = 128                   # partitions / edge slots per block
H = 64                    # half-block granule (edge slots)
W = 32                    # dst-window width (onehot cols per block)
BW = IN_SIZE + W          # block slab width (msg cols + selection cols)
GW = 8                    # windows per slab DMA chunk
WPG = P // W              # windows per output partition group (4)

F32 = mybir.dt.float32
BF16 = mybir.dt.bfloat16


def _np_bf16():
    import ml_dtypes
    return ml_dtypes.bfloat16


def _host_prep(inp, src, dst, edge_val):
    """Bucket edges per (core, window, rel); build half-block-packed slabs."""
    n_win = math.ceil(NPC / W)
    ncell = n_win * N_REL
    srcf = src.reshape(-1).astype(np.int64)
    dstf = dst.reshape(-1).astype(np.int64)
    valf = edge_val.reshape(-1).astype(np.float32)
    rel = np.repeat(np.arange(N_REL, dtype=np.int64), src.shape[1])

    core = dstf // NPC
    dloc = dstf % NPC
    win = dloc // W
    wloc = dloc % W
    cell = win * N_REL + rel
    key = core * ncell + cell

    counts = np.bincount(key, minlength=N_CORES * ncell).reshape(
        N_CORES, ncell)
    # BISECT: whole blocks only (even half counts) — no boundary halves.
    nh = (2 * np.maximum((counts.max(axis=0) + P - 1) // P, 1)).astype(
        np.int64)

    # Cumulative half starts, padded to an even half at each GW chunk
    # boundary so DMA chunks cover whole blocks.
    hs = np.zeros(ncell + 1, dtype=np.int64)
    off = 0
    for w0 in range(0, n_win, GW):
        w1 = min(w0 + GW, n_win)
        for c in range(w0 * N_REL, w1 * N_REL):
            hs[c] = off
            off += int(nh[c])
        off += off % 2  # pad to even at chunk end
    hs[ncell] = off
    T = off // 2  # blocks

    edt = _np_bf16()
    slab_all = np.zeros((N_CORES, P, T, BW), dtype=edt)

    order = np.argsort(key, kind="stable")
    grp_start = np.zeros(N_CORES * ncell, dtype=np.int64)
    np.cumsum(counts.reshape(-1)[:-1], out=grp_start[1:])
    j = np.arange(len(order), dtype=np.int64) - grp_start[key[order]]
    half = hs[cell[order]] + j // H
    p_row = (half % 2) * H + (j % H)
    t_col = half // 2
    c_ord = core[order]
    slab_all[c_ord, p_row, t_col, :IN_SIZE] = inp[srcf[order]].astype(edt)
    slab_all[c_ord, p_row, t_col, IN_SIZE + wloc[order]] = valf[order].astype(edt)

    return n_win, nh, hs, T, slab_all


_PROG_CACHE = {}


def _build_program(n_win, hs, T):
    key = (W, H, n_win, tuple(int(s) for s in hs), T)
    if key in _PROG_CACHE:
        return _PROG_CACHE[key]

    ncell = n_win * N_REL
    nc = bacc.Bacc("TRN2", target_bir_lowering=False, debug=False,
                   num_devices=N_CORES)
    wcat = nc.dram_tensor("wcat", [IN_SIZE, N_REL * OUT_SIZE], BF16,
                          kind="ExternalInput").ap()
    eslab = nc.dram_tensor("eslab", [P, T * BW], BF16,
                           kind="ExternalInput").ap()
    n_wcol = (n_win + WPG - 1) // WPG
    out = nc.dram_tensor("out", [P, n_wcol * OUT_SIZE], F32,
                         kind="ExternalOutput").ap()

    with tile.TileContext(nc) as tc, ExitStack() as ctx:
        p_w = ctx.enter_context(tc.tile_pool(name="p_w", bufs=1))
        p_msg = ctx.enter_context(tc.tile_pool(name="p_msg", bufs=3))
        p_agg = ctx.enter_context(tc.tile_pool(name="p_agg", bufs=2))
        p_c = ctx.enter_context(tc.tile_pool(name="p_c", bufs=1))
        ps_agg = ctx.enter_context(tc.tile_pool(name="ps_agg", bufs=4,
                                                space="PSUM"))
        ps_out = ctx.enter_context(tc.tile_pool(name="ps_out", bufs=2,
                                                space="PSUM"))

        wt = p_w.tile([IN_SIZE, N_REL * OUT_SIZE], BF16)
        nc.sync.dma_start(wt[:], wcat[:])
        outsb = p_c.tile([P, n_wcol * OUT_SIZE], F32)
        tail = n_win % WPG
        if tail:
            nc.vector.memset(
                outsb[tail * W:P, (n_win // WPG) * OUT_SIZE:], 0.0)

        bg_max = max(
            int(hs[min(w0 + GW, n_win) * N_REL
                   if w0 + GW < n_win else ncell] // 2
                - hs[w0 * N_REL] // 2)
            for w0 in range(0, n_win, GW))

        aggG = None
        for gi, w0 in enumerate(range(0, n_win, GW)):
            w1 = min(w0 + GW, n_win)
            hend = int(hs[w1 * N_REL]) if w1 < n_win else int(hs[ncell])
            t0, t1 = int(hs[w0 * N_REL]) // 2, hend // 2
            bg = t1 - t0
            mt = p_msg.tile([P, bg_max * BW], BF16, tag="msg")
            deng = nc.sync if gi % 2 == 0 else nc.scalar
            deng.dma_start(mt[:, :bg * BW], eslab[:, t0 * BW:t1 * BW])
            for w in range(w0, w1):
                g4 = w % WPG
                if g4 == 0:
                    aggG = p_agg.tile([IN_SIZE, WPG * P], BF16, tag="agg")
                ps = ps_agg.tile([IN_SIZE, N_REL * W], F32)
                segs_by_cell = []
                for r in range(N_REL):
                    c2 = w * N_REL + r
                    h0, h1 = int(hs[c2]), int(hs[c2 + 1])
                    fb0, fb1 = (h0 + 1) // 2, h1 // 2
                    segs = [(t, 0, P) for t in range(fb0, fb1)]
                    if h0 % 2:
                        segs.append((h0 // 2, H, H))
                    if h1 % 2:
                        segs.append((h1 // 2, 0, H))
                    segs_by_cell.append(segs)
                # Emit full blocks first, then boundary halves (fewer PE
                # tiling-mode switches); per-cell order is fulls->halves so
                # start/stop flags are first/last of that list.
                for pass_full in (True, False):
                    for r in range(N_REL):
                        segs = segs_by_cell[r]
                        for i, (t, pb, k) in enumerate(segs):
                            if (k == P) != pass_full:
                                continue
                            tb = t - t0
                            nc.tensor.matmul(
                                out=ps[:, r * W:(r + 1) * W],
                                lhsT=mt[pb:pb + k,
                                        tb * BW:tb * BW + IN_SIZE],
                                rhs=mt[pb:pb + k,
                                       tb * BW + IN_SIZE:(tb + 1) * BW],
                                start=(i == 0), stop=(i == len(segs) - 1))
                # aggG column layout: [rel][win-in-group][dst] so stage-2
                # lhsT slices are contiguous (walrus: weights AP must have
                # a single free dimension).
                aggG4 = aggG[:].rearrange(
                    "i (r g d) -> i r g d", r=N_REL, g=WPG, d=W)
                nc.scalar.copy(
                    aggG4[:, :, g4, :],
                    ps[:].rearrange("i (r d) -> i r d", r=N_REL, d=W))
                if g4 == WPG - 1 or w == n_win - 1:
                    nwg = g4 + 1
                    cg = w // WPG
                    po = ps_out.tile([nwg * W, OUT_SIZE], F32)
                    for r in range(N_REL):
                        lhs = aggG[:, r * WPG * W:r * WPG * W + nwg * W]
                        nc.tensor.matmul(
                            out=po[:],
                            lhsT=lhs,
                            rhs=wt[:, r * OUT_SIZE:(r + 1) * OUT_SIZE],
                            start=(r == 0), stop=(r == N_REL - 1))
                    nc.scalar.copy(
                        outsb[0:nwg * W, cg * OUT_SIZE:(cg + 1) * OUT_SIZE],
                        po[:])
        nc.sync.dma_start(out[:], outsb[:])

    nc.compile()
    _PROG_CACHE[key] = nc
    return nc


def kernel(inp, src, dst, edge_val, weights, bias):
    inp = np.asarray(inp, dtype=np.float32)
    src = np.asarray(src)
    dst = np.asarray(dst)
    edge_val = np.asarray(edge_val, dtype=np.float32)
    weights = np.asarray(weights, dtype=np.float32)
    bias = np.asarray(bias, dtype=np.float32)

    n_win, nh, hs, T, slab_all = _host_prep(inp, src, dst, edge_val)
    nc = _build_program(n_win, hs, T)

    edt = _np_bf16()
    wcat = np.ascontiguousarray(
        weights.transpose(1, 0, 2).reshape(IN_SIZE, N_REL * OUT_SIZE)
    ).astype(edt)

    in_maps = []
    for c in range(N_CORES):
        in_maps.append({
            "wcat": wcat,
            "eslab": slab_all[c].reshape(P, T * BW),
        })
    res = run_bass_kernel_spmd(nc, in_maps, list(range(N_CORES)))
    n_wcol = (n_win + WPG - 1) // WPG
    parts = []
    for c in range(N_CORES):
        arr = res.results[c]["out"].reshape(WPG, W, n_wcol, OUT_SIZE)
        nodes = arr.transpose(2, 0, 1, 3).reshape(n_wcol * P, OUT_SIZE)
        parts.append(nodes[:NPC])
    out = np.concatenate(parts, axis=0)
    out = out + bias.sum(axis=0)
    return out.astype(np.float32)


# revision 15
# speedup vs baseline: 4.3592x; 1.4934x over previous
"""GCN layer (4-relation message passing) on 8 Trainium2 NeuronCores.

out = sum_r (A_r @ inp) @ W_r + sum_r b_r,  A_r in COO form (dst, src, val).

Sharding: edges sharded by dst range; core c owns dst in [c*12500, (c+1)*12500).
Host stages a block slab (pure placement of input values - no host
arithmetic). Edges are bucketed per (window of 32 dst nodes, relation) cell;
each cell occupies ceil(maxcount/64) half-blocks of 64 edge slots, packed
contiguously, so two cells may share one 128-partition block. Normal blocks
are 96 columns: 0:64 message rows inp[src], 64:96 selection matrix
O[p, j] = val_p * (j == dstloc_p). Shared boundary blocks are 128 columns
with TWO selection groups (cols 64:96 for the even-half cell, 96:128 for the
odd-half cell; each group is zero on the other cell's rows), so every matmul
stays a full 128-row contraction.

Device, per cell, per touched block: PE accumulates
  psum[64in, 4*32][:, r*32:] += MSG^T @ O_grp.
Per 4 windows: one DVE copy psum->aggG bf16 [64, 4*128] (rel-major);
stage-2 po[4*32 dst, 64 outf] += aggG_r^T @ W_r (4 matmuls); one DVE copy
po -> outsb. Host: concat core slices, add summed bias.
"""

import math
from contextlib import ExitStack

import numpy as np

import concourse.bass as bass
import concourse.tile as tile
from concourse import bacc, mybir
from concourse.bass_utils import run_bass_kernel_spmd

# problem constants
N_NODES = 100000
N_REL = 4
N_EDGES = 1600000
IN_SIZE = 64
OUT_SIZE = 64

N_CORES = 8
NPC = N_NODES // N_CORES  # nodes (dst) per core
P = 128                   # partitions / edge slots per block
H = 64                    # half-block granule (edge slots)
W = 32                    # dst-window width (onehot cols per group)
GW = 8                    # windows per slab DMA chunk
WPG = P // W              # windows per output partition group (4)

F32 = mybir.dt.float32
BF16 = mybir.dt.bfloat16


def _np_bf16():
    import ml_dtypes
    return ml_dtypes.bfloat16


def _layout(counts_max, n_win):
    """Half-block packing shared by host prep and program build.

    Returns hs (per-cell half starts), cell_of_half, boundary flags and
    per-block column starts cs (cs[T] = total slab columns).
    """
    ncell = n_win * N_REL
    nh = np.maximum((counts_max + H - 1) // H, 1).astype(np.int64)
    hs = np.zeros(ncell + 1, dtype=np.int64)
    halves = []
    off = 0
    for w0 in range(0, n_win, GW):
        w1 = min(w0 + GW, n_win)
        for c in range(w0 * N_REL, w1 * N_REL):
            hs[c] = off
            halves.extend([c] * int(nh[c]))
            off += int(nh[c])
        if off % 2:
            halves.append(-1)  # pad half
            off += 1
    hs[ncell] = off
    T = off // 2
    coh = np.asarray(halves, dtype=np.int64).reshape(T, 2)
    boundary = (coh[:, 0] != coh[:, 1]) & (coh[:, 0] >= 0) & (coh[:, 1] >= 0)
    wb = np.where(boundary, IN_SIZE + 2 * W, IN_SIZE + W).astype(np.int64)
    cs = np.zeros(T + 1, dtype=np.int64)
    np.cumsum(wb, out=cs[1:])
    return hs, boundary, cs, T


def _host_prep(inp, src, dst, edge_val):
    n_win = math.ceil(NPC / W)
    ncell = n_win * N_REL
    srcf = src.reshape(-1).astype(np.int64)
    dstf = dst.reshape(-1).astype(np.int64)
    valf = edge_val.reshape(-1).astype(np.float32)
    rel = np.repeat(np.arange(N_REL, dtype=np.int64), src.shape[1])

    core = dstf // NPC
    dloc = dstf % NPC
    win = dloc // W
    wloc = dloc % W
    cell = win * N_REL + rel
    key = core * ncell + cell

    counts = np.bincount(key, minlength=N_CORES * ncell).reshape(
        N_CORES, ncell)
    hs, boundary, cs, T = _layout(counts.max(axis=0), n_win)

    edt = _np_bf16()
    tot = int(cs[T])
    slab_all = np.zeros((N_CORES, P, tot), dtype=edt)

    order = np.argsort(key, kind="stable")
    grp_start = np.zeros(N_CORES * ncell, dtype=np.int64)
    np.cumsum(counts.reshape(-1)[:-1], out=grp_start[1:])
    j = np.arange(len(order), dtype=np.int64) - grp_start[key[order]]
    half = hs[cell[order]] + j // H
    p_row = (half % 2) * H + (j % H)
    t_col = half // 2
    c_ord = core[order]
    grp = ((half % 2) == 1) & boundary[t_col]
    msg_cols = cs[t_col][:, None] + np.arange(IN_SIZE)[None, :]
    slab_all[c_ord[:, None], p_row[:, None], msg_cols] = \
        inp[srcf[order]].astype(edt)
    oh_col = cs[t_col] + IN_SIZE + grp * W + wloc[order]
    slab_all[c_ord, p_row, oh_col] = valf[order].astype(edt)

    return n_win, hs, boundary, cs, T, slab_all


_PROG_CACHE = {}


def _build_program(n_win, hs, boundary, cs, T):
    key = (W, H, n_win, tuple(int(s) for s in hs), tuple(int(b) for b in boundary))
    if key in _PROG_CACHE:
        return _PROG_CACHE[key]

    ncell = n_win * N_REL
    tot = int(cs[T])
    nc = bacc.Bacc("TRN2", target_bir_lowering=False, debug=False,
                   num_devices=N_CORES)
    wcat = nc.dram_tensor("wcat", [IN_SIZE, N_REL * OUT_SIZE], BF16,
                          kind="ExternalInput").ap()
    eslab = nc.dram_tensor("eslab", [P, tot], BF16,
                           kind="ExternalInput").ap()
    n_wcol = (n_win + WPG - 1) // WPG
    out = nc.dram_tensor("out", [P, n_wcol * OUT_SIZE], F32,
                         kind="ExternalOutput").ap()

    with tile.TileContext(nc) as tc, ExitStack() as ctx:
        p_w = ctx.enter_context(tc.tile_pool(name="p_w", bufs=1))
        p_msg = ctx.enter_context(tc.tile_pool(name="p_msg", bufs=4))
        p_agg = ctx.enter_context(tc.tile_pool(name="p_agg", bufs=2))
        p_c = ctx.enter_context(tc.tile_pool(name="p_c", bufs=1))
        ps_agg = ctx.enter_context(tc.tile_pool(name="ps_agg", bufs=6,
                                                space="PSUM"))
        ps_out = ctx.enter_context(tc.tile_pool(name="ps_out", bufs=2,
                                                space="PSUM"))

        wt = p_w.tile([IN_SIZE, N_REL * OUT_SIZE], BF16)
        nc.sync.dma_start(wt[:], wcat[:])
        outsb = p_c.tile([P, n_wcol * OUT_SIZE], F32)
        tail = n_win % WPG
        if tail:
            nc.vector.memset(
                outsb[tail * W:P, (n_win // WPG) * OUT_SIZE:], 0.0)

        def grp_end_t(w1):
            return int(hs[w1 * N_REL]) // 2 if w1 < n_win else T

        cmax = max(
            int(cs[grp_end_t(min(w0 + GW, n_win))] - cs[int(hs[w0 * N_REL]) // 2])
            for w0 in range(0, n_win, GW))

        chunks = []
        for w0 in range(0, n_win, GW):
            w1 = min(w0 + GW, n_win)
            t0, t1 = int(hs[w0 * N_REL]) // 2, grp_end_t(w1)
            chunks.append((w0, w1, int(cs[t0]), int(cs[t1])))

        PF = 3
        mts = {}

        def issue(k):
            if k >= len(chunks):
                return
            _, _, c0, c1 = chunks[k]
            mt = p_msg.tile([P, cmax], BF16, tag="msg")
            deng = nc.sync if k % 2 == 0 else nc.scalar
            deng.dma_start(mt[:, :c1 - c0], eslab[:, c0:c1])
            mts[k] = mt

        for k in range(PF):
            issue(k)

        aggG = None
        for gi, (w0, w1, c0, c1) in enumerate(chunks):
            mt = mts.pop(gi)
            issue(gi + PF)
            for w in range(w0, w1):
                g4 = w % WPG
                if g4 == 0:
                    aggG = p_agg.tile([IN_SIZE, WPG * P], BF16, tag="agg")
                ps = ps_agg.tile([IN_SIZE, N_REL * W], F32)
                for r in range(N_REL):
                    c2 = w * N_REL + r
                    h0, h1 = int(hs[c2]), int(hs[c2 + 1])
                    tA, tB = h0 // 2, (h1 - 1) // 2
                    for i, t in enumerate(range(tA, tB + 1)):
                        g = 1 if (t == tA and h0 % 2 == 1
                                  and boundary[t]) else 0
                        csb = int(cs[t]) - c0
                        nc.tensor.matmul(
                            out=ps[:, r * W:(r + 1) * W],
                            lhsT=mt[:, csb:csb + IN_SIZE],
                            rhs=mt[:, csb + IN_SIZE + g * W:
                                   csb + IN_SIZE + (g + 1) * W],
                            start=(i == 0), stop=(t == tB))
                aggG4 = aggG[:].rearrange(
                    "i (r g d) -> i r g d", r=N_REL, g=WPG, d=W)
                nc.scalar.copy(
                    aggG4[:, :, g4, :],
                    ps[:].rearrange("i (r d) -> i r d", r=N_REL, d=W))
                if g4 == WPG - 1 or w == n_win - 1:
                    nwg = g4 + 1
                    cg = w // WPG
                    po = ps_out.tile([nwg * W, OUT_SIZE], F32)
                    for r in range(N_REL):
                        lhs = aggG[:, r * WPG * W:r * WPG * W + nwg * W]
                        nc.tensor.matmul(
                            out=po[:],
                            lhsT=lhs,
                            rhs=wt[:, r * OUT_SIZE:(r + 1) * OUT_SIZE],
                            start=(r == 0), stop=(r == N_REL - 1))
                    nc.scalar.copy(
                        outsb[0:nwg * W, cg * OUT_SIZE:(cg + 1) * OUT_SIZE],
                        po[:])
                    if cg == n_wcol // 2:
                        nc.sync.dma_start(
                            out[:, :(n_wcol // 2 + 1) * OUT_SIZE],
                            outsb[:, :(n_wcol // 2 + 1) * OUT_SIZE])
        nc.sync.dma_start(out[:, (n_wcol // 2 + 1) * OUT_SIZE:],
                          outsb[:, (n_wcol // 2 + 1) * OUT_SIZE:])

    nc.compile()
    _PROG_CACHE[key] = nc
    return nc


def kernel(inp, src, dst, edge_val, weights, bias):
    inp = np.asarray(inp, dtype=np.float32)
    src = np.asarray(src)
    dst = np.asarray(dst)
    edge_val = np.asarray(edge_val, dtype=np.float32)
    weights = np.asarray(weights, dtype=np.float32)
    bias = np.asarray(bias, dtype=np.float32)

    n_win, hs, boundary, cs, T, slab_all = _host_prep(
        inp, src, dst, edge_val)
    nc = _build_program(n_win, hs, boundary, cs, T)

    edt = _np_bf16()
    wcat = np.ascontiguousarray(
        weights.transpose(1, 0, 2).reshape(IN_SIZE, N_REL * OUT_SIZE)
    ).astype(edt)

    in_maps = []
    for c in range(N_CORES):
        in_maps.append({
            "wcat": wcat,
            "eslab": slab_all[c],
        })
    res = run_bass_kernel_spmd(nc, in_maps, list(range(N_CORES)))
    n_wcol = (n_win + WPG - 1) // WPG
    parts = []
    for c in range(N_CORES):
        arr = res.results[c]["out"].reshape(WPG, W, n_wcol, OUT_SIZE)
        nodes = arr.transpose(2, 0, 1, 3).reshape(n_wcol * P, OUT_SIZE)
        parts.append(nodes[:NPC])
    out = np.concatenate(parts, axis=0)
    out = out + bias.sum(axis=0)
    return out.astype(np.float32)
